# revision 1
# baseline (speedup 1.0000x reference)
"""Trainium2 Bass kernel for nn_Attention_4037269258732 (GQA attention with
RoPE, causal mask, and per-head sink-logit LSE renormalization).

Problem:  B=1, S=2048, DIM=2048, H=32 q-heads, KVH=8 kv-heads, HD=64.
          out = Wo @ attn(RoPE(Wq x), RoPE(Wk x), Wv x) + bo, causal,
          with out rows scaled by sigmoid(lse - sink_h).

Sharding (8 cores, tensor-parallel over heads):
  core c owns q-heads [4c, 4c+4), kv-head c, the matching rows of
  wq/wk/wv, wo's input-dim slice [256c, 256c+256), and sinks[4c:4c+4].
  Each core computes a full-shape [S, DIM] partial of the output
  projection (wo_b/8 added on every core); the host sums the 8 partials
  (that sum is the o-dim contraction of the output projection).

Device dataflow per core (feature dims on SBUF partitions so every
matmul chains without transposes; bf16 matmul operands / fp32 PSUM):
  qT[256,S], kT[64,S], vT[64,S] = W.T @ xT      (xT host-transposed)
  RoPE fused into PSUM eviction: q = (q+b)*cos + shifted(q+b)*sin_rot
  (rot_half as 32-partition-shifted DVE reads; sign folded into sin_rot)
  v_nat[S,64] via PE transpose;  Vext = [v_nat | 1]
  per (block b of 512 sq, sk-tile t, head h):
    P^T[sk,sq] = exp(kT_t.T @ q / 8)   (tri-mask on diagonal subtiles,
                                        upper-right tiles skipped)
    outT_ext[65,sq] += Vext_t.T @ P^T  (row 64 = sum_exp = softmax denom)
  per block: renorm rows r=sum_exp+e^sink at partitions {0,32,64,96},
    1/r via exp(-ln r) on ACT, broadcast via K=1 matmul,
    out_norm = outT * bcast;  then final[sq_tile, :] = outstk.T @ woT
    + wo_b/8 -> DRAM partial
"""

import numpy as np
import ml_dtypes

import bass_rust
import concourse.bass as bass
import concourse.tile as tile
from concourse import mybir
from concourse.bass_utils import run_bass_kernel_spmd

F32 = mybir.dt.float32
BF16 = mybir.dt.bfloat16
AF = mybir.ActivationFunctionType
OP = mybir.AluOpType
BF = ml_dtypes.bfloat16

B, S, DIM = 1, 2048, 2048
H, KVH, HD = 32, 8, 64
NCORES = 8
QH = H // NCORES          # 4 q heads per core
SBLK = 512                # sq block size
NSB = S // SBLK           # 4
NDC = DIM // 128          # 16 contraction chunks
NST = S // 128            # 16 sk tiles
SCALE = 1.0 / 8.0         # 1/sqrt(HD)

_ws_ctr = [0]


def _fix_range_clears(nc):
    """walrus here rejects the EVENT_SEMAPHORE_RANGE_CLEAR ISA struct
    ("ISA wrong length"); replace with per-sem write-0 NoOps."""
    import re as _re
    for f in nc.m.functions:
        for blk in f.blocks:
            out, changed = [], False
            for inst in blk.instructions:
                if type(inst).__name__ == "InstISA" and inst.isa_opcode == 176:
                    m = _re.search(r"range_first=(\d+) range_last=(\d+)", inst.concise())
                    first, last = int(m.group(1)), int(m.group(2))
                    for semid in range(first, last + 1):
                        _ws_ctr[0] += 1
                        nop = mybir.InstNoOp(name=f"I-rc-{_ws_ctr[0]}", ins=[], outs=[])
                        nop.engine = inst.engine
                        nop.sync_info = bass_rust.SyncInfo(
                            on_wait=[],
                            on_update=[
                                bass_rust.SyncUpdate(
                                    sync_type="semaphore",
                                    id=semid,
                                    update_mode="sem-wr-imm",
                                    update_value=0,
                                )
                            ],
                        )
                        out.append(nop)
                    changed = True
                    continue
                out.append(inst)
            if changed:
                blk.instructions = out


def _split_excess_waits(nc, max_waits=1):
    """walrus on this image encodes at most one SyncWait per instruction;
    hoist excess waits onto same-engine NoOps placed just before."""
    for f in nc.m.functions:
        for blk in f.blocks:
            out, changed = [], False
            for inst in blk.instructions:
                si = inst.sync_info
                waits = list(si.on_wait) if si is not None else []
                if len(waits) > max_waits:
                    excess, keep = waits[:-max_waits], waits[-max_waits:]
                    for k in range(0, len(excess), max_waits):
                        _ws_ctr[0] += 1
                        nop = mybir.InstNoOp(name=f"I-ws-{_ws_ctr[0]}", ins=[], outs=[])
                        nop.engine = inst.engine
                        nop.sync_info = bass_rust.SyncInfo(
                            on_wait=excess[k : k + max_waits], on_update=[]
                        )
                        out.append(nop)
                    inst.sync_info = bass_rust.SyncInfo(
                        on_wait=keep, on_update=list(si.on_update)
                    )
                    changed = True
                out.append(inst)
            if changed:
                blk.instructions = out


def prep_inputs(inputs):
    """Host-side sharding/layout prep. Returns per-core input maps."""
    x = np.asarray(inputs["x"], np.float32)
    rope = np.asarray(inputs["rope_cache"], np.float32)
    wq = np.asarray(inputs["wq_w"], np.float32)
    bq = np.asarray(inputs["wq_b"], np.float32)
    wk = np.asarray(inputs["wk_w"], np.float32)
    bk = np.asarray(inputs["wk_b"], np.float32)
    wv = np.asarray(inputs["wv_w"], np.float32)
    bv = np.asarray(inputs["wv_b"], np.float32)
    wo = np.asarray(inputs["wo_w"], np.float32)
    bo = np.asarray(inputs["wo_b"], np.float32)
    sinks = np.asarray(inputs["sinks"], np.float32)

    xT = np.ascontiguousarray(x[0].T).astype(BF)            # [DIM, S]
    cosT = rope[:, :HD].T                                   # [64, S]
    sinT = rope[:, HD:].T
    cos2 = np.ascontiguousarray(np.concatenate([cosT, cosT], 0)).astype(np.float32)
    # sin_rot indexed by SOURCE partition: source rows hd in [0,32) land at
    # out rows hd+32 with +sin[hd+32]; source rows hd in [32,64) land at
    # out rows hd-32 with -sin[hd-32]. Duplicated for both heads per tile.
    sr = np.concatenate([sinT[32:64], -sinT[0:32]], 0)      # [64, S]
    sin_rot2 = np.ascontiguousarray(np.concatenate([sr, sr], 0)).astype(np.float32)
    tri = np.triu(np.ones((128, 128), BF))                  # mask[p, j] = j >= p
    ident = np.eye(HD, dtype=BF)
    wob8 = (bo / NCORES).reshape(1, DIM).astype(np.float32)

    in_maps = []
    for c in range(NCORES):
        qs = slice(c * QH * HD, (c + 1) * QH * HD)          # 256 q rows
        ks = slice(c * HD, (c + 1) * HD)                    # 64 kv rows
        # wproj columns: [q 256 | k 64 | v 64] = 384
        wproj = np.concatenate([wq[qs].T, wk[ks].T, wv[ks].T], axis=1)
        bcol = np.zeros((128, 3), np.float32)
        bcol[:, 0] = bq[qs][0:128]
        bcol[:, 1] = bq[qs][128:256]
        bcol[0:64, 2] = bk[ks]
        bcol[64:128, 2] = bv[ks]
        woT = np.ascontiguousarray(wo[:, qs].T).astype(BF)  # [256, DIM]
        esink = np.tile(np.exp(sinks[c * QH : (c + 1) * QH]).reshape(1, QH),
                        (128, 1))
        in_maps.append(
            {
                "xT": xT,
                "wproj": np.ascontiguousarray(wproj).astype(BF),
                "bproj": bcol,
                "cos2": cos2,
                "sinr2": sin_rot2,
                "woT": woT,
                "wob8": wob8,
                "esink": esink.astype(np.float32),
                "tri": tri,
                "ident": ident,
                "ones_f": np.ones((128, 128), np.float32),
                "onesb": np.ones((128, 1), BF),
            }
        )
    return in_maps


def build_nc(split_waits=True):
    nc = bass.Bass("TRN2", target_bir_lowering=False, debug=False, num_devices=NCORES)
    xT = nc.dram_tensor("xT", [DIM, S], BF16, kind="ExternalInput").ap()
    wproj = nc.dram_tensor("wproj", [DIM, 384], BF16, kind="ExternalInput").ap()
    bproj = nc.dram_tensor("bproj", [128, 3], F32, kind="ExternalInput").ap()
    cos2 = nc.dram_tensor("cos2", [128, S], F32, kind="ExternalInput").ap()
    sinr2 = nc.dram_tensor("sinr2", [128, S], F32, kind="ExternalInput").ap()
    woT = nc.dram_tensor("woT", [2 * 128, DIM], BF16, kind="ExternalInput").ap()
    wob8 = nc.dram_tensor("wob8", [1, DIM], F32, kind="ExternalInput").ap()
    esink = nc.dram_tensor("esink", [128, QH], F32, kind="ExternalInput").ap()
    tri = nc.dram_tensor("tri", [128, 128], BF16, kind="ExternalInput").ap()
    ident = nc.dram_tensor("ident", [HD, HD], BF16, kind="ExternalInput").ap()
    ones_f = nc.dram_tensor("ones_f", [128, 128], F32, kind="ExternalInput").ap()
    onesb = nc.dram_tensor("onesb", [128, 1], BF16, kind="ExternalInput").ap()
    out = nc.dram_tensor("out", [S, DIM], F32, kind="ExternalOutput").ap()

    with tile.TileContext(nc) as tc:
        with tc.tile_pool(name="persist", bufs=1) as P:
            # ---- long-lived tiles ----
            esink_t = P.tile([128, QH], F32, tag="esink")
            tri_t = P.tile([128, 128], BF16, tag="tri")
            wo_t = [
                P.tile([128, DIM], BF16, name=f"wo{i}", tag=f"wo{i}")
                for i in range(2)
            ]
            biasb = P.tile([128, DIM], F32, tag="biasb")
            ones_ft = P.tile([128, 128], F32, tag="ones_ft")
            wob_row = P.tile([1, DIM], F32, tag="wobrow")
            # tiny dummy Exp/Ln to pull the ACT table load off the
            # attention critical path
            scr = P.tile([1, 16], F32, tag="scr")
            qp = [P.tile([128, S], BF16, name=f"qp{i}", tag=f"qp{i}") for i in range(2)]
            kT2 = P.tile([128, S], BF16, tag="kT2")
            vext = P.tile([128, NST * (HD + 1)], BF16, tag="vext")
            onesb_t = P.tile([128, 1], BF16, tag="onesb_t")
            outstk = [P.tile([128, S], BF16, name=f"os{i}", tag=f"os{i}") for i in range(2)]
            vT = P.tile([64, S], BF16, tag="vT")
            idp_t = P.tile([HD, HD], BF16, tag="idp")

            # ---- qkv projection, rope fused into eviction ----
            with (
                tc.tile_pool(name="projw", bufs=1) as PW,
                tc.tile_pool(name="tmp", bufs=2) as TMP,
                tc.tile_pool(name="psproj", bufs=2, space="PSUM") as PSP,
                tc.tile_pool(name="psv", bufs=2, space="PSUM") as PSV,
            ):
                x_t, w_t = [], []
                for dc in range(NDC):
                    wt = PW.tile([128, 384], BF16, name=f"w{dc}", tag=f"w{dc}")
                    nc.gpsimd.dma_start(wt[:], wproj[dc * 128 : (dc + 1) * 128, :])
                    w_t.append(wt)
                    xt = PW.tile([128, S], BF16, name=f"x{dc}", tag=f"x{dc}")
                    if dc < 2:
                        for q4 in range(4):
                            nc.sync.dma_start(
                                xt[:, q4 * SBLK : (q4 + 1) * SBLK],
                                xT[dc * 128 : (dc + 1) * 128,
                                   q4 * SBLK : (q4 + 1) * SBLK],
                            )
                    else:
                        nc.sync.dma_start(xt[:], xT[dc * 128 : (dc + 1) * 128, :])
                    x_t.append(xt)
                bcol_t = PW.tile([128, 3], F32, tag="bcol")
                nc.gpsimd.dma_start(bcol_t[:], bproj[:])
                cos_t = PW.tile([128, S], F32, tag="cos")
                nc.gpsimd.dma_start(cos_t[:], cos2[:])
                sinr_t = PW.tile([128, S], F32, tag="sinr")
                nc.gpsimd.dma_start(sinr_t[:], sinr2[:])
                id_t = PW.tile([HD, HD], BF16, tag="ident")
                nc.gpsimd.dma_start(id_t[:], ident[:])
                nc.gpsimd.dma_start(onesb_t[:], onesb[:])
                nc.gpsimd.dma_start(esink_t[:], esink[:])
                nc.gpsimd.dma_start(tri_t[:], tri[:])
                nc.gpsimd.dma_start(ones_ft[:], ones_f[:])
                nc.gpsimd.dma_start(wob_row[:], wob8[:])
                for i in range(2):
                    nc.gpsimd.dma_start(
                        wo_t[i][:], woT[i * 128 : (i + 1) * 128, :]
                    )
                nc.gpsimd.dma_start(idp_t[:], ident[:])
                nc.scalar.activation(scr[0:1, 0:3], bcol_t[0:1, 0:3], AF.Exp)
                nc.scalar.activation(scr[0:1, 0:3], scr[0:1, 0:3], AF.Ln)

                for sb in range(NSB):
                    ss = slice(sb * SBLK, (sb + 1) * SBLK)
                    ps = [
                        PSP.tile([128, SBLK], F32, name=f"pp{j}", tag=f"pp{j}")
                        for j in range(3)
                    ]
                    for dc in range(NDC):
                        for j, (c0, c1) in enumerate(
                            [(0, 128), (128, 256), (256, 384)]
                        ):
                            nc.tensor.matmul(
                                ps[j][:],
                                w_t[dc][:, c0:c1],
                                x_t[dc][:, ss],
                                start=(dc == 0),
                                stop=(dc == NDC - 1),
                            )
                    # rope eviction: cos part for both heads of a ptile at
                    # once; rot part via 32-partition-shifted reads with the
                    # sign folded into sinr_t; combine per head into qh (bf16)
                    for i in range(2):
                        t1 = TMP.tile([128, SBLK], F32, name="t1", tag="t1")
                        nc.vector.scalar_tensor_tensor(
                            t1[:], ps[i][:], bcol_t[:, i : i + 1], cos_t[:, ss],
                            op0=OP.add, op1=OP.mult,
                        )
                        t2 = TMP.tile([128, SBLK], F32, name="t2", tag="t2")
                        for g in range(4):
                            d0 = 32 * g
                            s0 = 32 * g + 32 if g % 2 == 0 else 32 * g - 32
                            nc.vector.scalar_tensor_tensor(
                                t2[d0 : d0 + 32, :],
                                ps[i][s0 : s0 + 32, :],
                                bcol_t[s0 : s0 + 32, i : i + 1],
                                sinr_t[s0 : s0 + 32, ss],
                                op0=OP.add, op1=OP.mult,
                            )
                        nc.vector.tensor_tensor(
                            qp[i][:, ss], t1[:], t2[:], op=OP.add
                        )
                    # k: rows 0:64 of ps[2]
                    tk1 = TMP.tile([64, SBLK], F32, name="tk1", tag="tk1")
                    nc.vector.scalar_tensor_tensor(
                        tk1[:], ps[2][0:64, :], bcol_t[0:64, 2:3], cos_t[0:64, ss],
                        op0=OP.add, op1=OP.mult,
                    )
                    tk2 = TMP.tile([64, SBLK], F32, name="tk2", tag="tk2")
                    nc.vector.scalar_tensor_tensor(
                        tk2[0:32, :], ps[2][32:64, :], bcol_t[32:64, 2:3],
                        sinr_t[32:64, ss], op0=OP.add, op1=OP.mult,
                    )
                    nc.vector.scalar_tensor_tensor(
                        tk2[32:64, :], ps[2][0:32, :], bcol_t[0:32, 2:3],
                        sinr_t[0:32, ss], op0=OP.add, op1=OP.mult,
                    )
                    nc.vector.tensor_tensor(
                        kT2[0:64, ss], tk1[:], tk2[:], op=OP.add
                    )
                    nc.vector.tensor_copy(kT2[64:128, ss], kT2[0:64, ss])
                    # v: rows 64:128 of ps[2], bias only
                    nc.vector.tensor_scalar_add(
                        vT[:, ss], ps[2][64:128, :], bcol_t[64:128, 2:3]
                    )
                    # transpose this block's v tiles into Vext right away
                    # (sb=3's transposes are deferred past attention block 0
                    # so the PE doesn't stall on the last rope eviction)
                    if sb < 3:
                        for t in range(4 * sb, 4 * sb + 4):
                            pv = PSV.tile([128, HD], BF16, name="pv", tag="pv")
                            nc.tensor.transpose(
                                pv[:], vT[:, t * 128 : (t + 1) * 128], id_t[:]
                            )
                            nc.vector.tensor_copy(
                                vext[:, t * 65 : t * 65 + 64], pv[:]
                            )
                            nc.vector.tensor_copy(
                                vext[:, t * 65 + 64 : t * 65 + 65], onesb_t[:]
                            )

            # ---- attention + per-block renorm + output projection ----
            with (
                tc.tile_pool(name="pss", bufs=2, space="PSUM") as PSS,
                tc.tile_pool(name="pso", bufs=1, space="PSUM") as PSO,
                tc.tile_pool(name="aux", bufs=2, space="PSUM") as AUX,
                tc.tile_pool(name="ptp", bufs=8) as PTP,
                tc.tile_pool(name="rows", bufs=2) as RP,
                tc.tile_pool(name="rbp", bufs=2) as RBP,
                tc.tile_pool(name="oev", bufs=4) as OE,
            ):
                # wo bias broadcast rows (K=1 matmuls)
                for db in range(NSB):
                    ds = slice(db * SBLK, (db + 1) * SBLK)
                    ps_bb = AUX.tile([128, SBLK], F32, name="ps_bb", tag="aux")
                    nc.tensor.matmul(
                        ps_bb[:], ones_ft[0:1, :], wob_row[0:1, ds],
                        start=True, stop=True,
                    )
                    nc.vector.tensor_copy(biasb[:, ds], ps_bb[:])
                for b in range(NSB):
                    pso = [
                        PSO.tile([65, SBLK], F32, name=f"oo{i}", tag=f"oo{i}")
                        for i in range(QH)
                    ]
                    nt = 4 * b + 4
                    for t in range(nt):
                        off = 128 * (t - 4 * b) if t >= 4 * b else 0
                        ptts = []
                        for hp in range(2):
                            # two K=64 score matmuls packed into disjoint
                            # PE row groups -> run concurrently
                            psa = PSS.tile([128, SBLK], F32, name="psa", tag="ss")
                            psb = PSS.tile([128, SBLK], F32, name="psb", tag="ss")
                            nc.tensor.matmul(
                                psa[:, off:SBLK],
                                kT2[0:64, t * 128 : (t + 1) * 128],
                                qp[hp][0:64, b * SBLK + off : (b + 1) * SBLK],
                                start=True,
                                stop=True,
                                tile_position=(0, 0),
                            )
                            nc.tensor.matmul(
                                psb[:, off:SBLK],
                                kT2[64:128, t * 128 : (t + 1) * 128],
                                qp[hp][64:128, b * SBLK + off : (b + 1) * SBLK],
                                start=True,
                                stop=True,
                                tile_position=(64, 0),
                            )
                            for lane, pss in ((0, psa), (1, psb)):
                                ptt = PTP.tile([128, SBLK], BF16, name="ptt", tag="pt")
                                nc.scalar.activation(
                                    ptt[:, off:SBLK], pss[:, off:SBLK], AF.Exp,
                                    scale=SCALE,
                                )
                                if t >= 4 * b:
                                    nc.vector.tensor_tensor(
                                        ptt[:, off : off + 128],
                                        ptt[:, off : off + 128],
                                        tri_t[:],
                                        op=OP.mult,
                                    )
                                ptts.append(ptt)
                        for h in range(QH):
                            nc.tensor.matmul(
                                pso[h][:, off:SBLK],
                                vext[:, t * 65 : (t + 1) * 65],
                                ptts[h][:, off:SBLK],
                                start=(t == 0),
                                stop=(t == nt - 1),
                            )
                    # sink renorm via exp(-ln r) + K=1 bcast matmul
                    rowb = RP.tile([128, SBLK], F32, name="rowb", tag="rowb")
                    nc.gpsimd.memset(rowb[:], 1.0)
                    for h in range(QH):
                        nc.vector.tensor_scalar_add(
                            rowb[32 * h : 32 * h + 1, :],
                            pso[h][64:65, :],
                            esink_t[64:65, h : h + 1],
                        )
                    rinvb = RP.tile([128, SBLK], F32, name="rinvb", tag="rinvb")
                    nc.scalar.activation(rinvb[:], rowb[:], AF.Ln)
                    nc.scalar.activation(rowb[:], rinvb[:], AF.Exp, scale=-1.0)
                    for h in range(QH):
                        qb = (h % 2) * 64
                        ps_rb = AUX.tile([64, SBLK], F32, name="ps_rb", tag="aux")
                        nc.tensor.matmul(
                            ps_rb[:], ones_ft[32 * h : 32 * h + 1, 0:64],
                            rowb[32 * h : 32 * h + 1, :],
                            start=True, stop=True,
                            tile_position=(32 * h, 0),
                        )
                        rb = RBP.tile([64, SBLK], F32, name="rb", tag="rb")
                        nc.vector.tensor_copy(rb[:], ps_rb[:])
                        nc.vector.tensor_tensor(
                            outstk[h // 2][qb : qb + 64, b * SBLK : (b + 1) * SBLK],
                            pso[h][0:64, :],
                            rb[:],
                            op=OP.mult,
                        )
                    # output projection for this block's 4 sq tiles
                    for st in range(4 * b, 4 * b + 4):
                        for db in range(NSB):
                            ds = slice(db * SBLK, (db + 1) * SBLK)
                            psf = AUX.tile([128, SBLK], F32, name="psf", tag="aux")
                            nc.tensor.matmul(
                                psf[:],
                                outstk[0][:, st * 128 : (st + 1) * 128],
                                wo_t[0][:, ds],
                                start=True,
                                stop=False,
                            )
                            nc.tensor.matmul(
                                psf[:],
                                outstk[1][:, st * 128 : (st + 1) * 128],
                                wo_t[1][:, ds],
                                start=False,
                                stop=True,
                            )
                            ot = OE.tile([128, SBLK], F32, name="ot", tag="oe")
                            nc.vector.tensor_tensor(
                                ot[:], psf[:], biasb[:, ds], op=OP.add
                            )
                            nc.sync.dma_start(
                                out[st * 128 : (st + 1) * 128, ds], ot[:]
                            )
                    if b == 0:
                        for t in range(12, 16):
                            pv2 = AUX.tile([128, HD], BF16, name="pv2", tag="aux")
                            nc.tensor.transpose(
                                pv2[:], vT[:, t * 128 : (t + 1) * 128], idp_t[:]
                            )
                            nc.vector.tensor_copy(
                                vext[:, t * 65 : t * 65 + 64], pv2[:]
                            )
                            nc.vector.tensor_copy(
                                vext[:, t * 65 + 64 : t * 65 + 65], onesb_t[:]
                            )

    _fix_range_clears(nc)
    if split_waits:
        _split_excess_waits(nc)
    return nc


_nc_cache = [None]


def kernel(**inputs):
    in_maps = prep_inputs(inputs)
    if _nc_cache[0] is None:
        _nc_cache[0] = build_nc()
    nc = _nc_cache[0]
    res = run_bass_kernel_spmd(nc, in_maps, list(range(NCORES)))
    acc = res.results[0]["out"].astype(np.float32)
    for i in range(1, NCORES):
        acc = acc + res.results[i]["out"]
    return acc.reshape(B, S, DIM)



# revision 11
# speedup vs baseline: 1.1769x; 1.1769x over previous
"""Trainium2 Bass kernel for nn_Attention_4037269258732 (GQA attention with
RoPE, causal mask, and per-head sink-logit LSE renormalization).

Problem:  B=1, S=2048, DIM=2048, H=32 q-heads, KVH=8 kv-heads, HD=64.
          out = Wo @ attn(RoPE(Wq x), RoPE(Wk x), Wv x) + bo, causal,
          with out rows scaled by r = sumexp/(sumexp + e^sink).

Sharding (8 cores, tensor-parallel over heads):
  core c owns q-heads [4c, 4c+4), kv-head c, the matching rows of
  wq/wk/wv, wo's input-dim slice [256c, 256c+256), and sinks[4c:4c+4].
  Each core computes a full-shape [S, DIM] bf16 partial of the output
  projection (no bias); the host sums the 8 partials and adds wo_b.

v2 layout: merged software pipeline of 5 rounds.  Round r interleaves, in
PE program order, the QKV projection of sequence block r with the
attention of block r-1, so the PE never idles long enough to trip the
HAM clock gate and the ACT-engine exp evictions overlap matmuls.

  - scores: per head-pair, two row-packed K=64 matmuls into one
    [128,1024] fp32 PSUM tile (2 banks); a single wide ACT Exp evicts
    both heads at once (amortizes the 352-cycle ACT fixed cost).
  - PV: col-tiled M=64 pairs (tile_position (0,0)/(0,64)) accumulate two
    heads into one PSUM bank concurrently; per-head softmax denominators
    via 4 concurrent M=1 matmuls (cols 0/32/64/96) into one bank.
  - PSUM budget (8 banks): S-ring 2x[128,1024] (4) shared by scores,
    V-transposes, renorm broadcasts and outproj accumulators; proj 1;
    pso 2; denom 1.
"""

import numpy as np
import ml_dtypes

import bass_rust
import concourse.bass as bass
import concourse.tile as tile
from concourse import mybir
from concourse.bass_utils import run_bass_kernel_spmd

F32 = mybir.dt.float32
BF16 = mybir.dt.bfloat16
AF = mybir.ActivationFunctionType
OP = mybir.AluOpType
BF = ml_dtypes.bfloat16

B, S, DIM = 1, 2048, 2048
H, KVH, HD = 32, 8, 64
NCORES = 8
QH = H // NCORES          # 4 q heads per core
SBLK = 512                # sq block size
NSB = S // SBLK           # 4
NDC = DIM // 128          # 16 contraction chunks
NST = S // 128            # 16 sk tiles
SCALE = 1.0 / 8.0         # 1/sqrt(HD)

_ws_ctr = [0]


def _fix_range_clears(nc):
    """walrus here rejects the EVENT_SEMAPHORE_RANGE_CLEAR ISA struct
    ("ISA wrong length"); replace with per-sem write-0 NoOps."""
    import re as _re
    for f in nc.m.functions:
        for blk in f.blocks:
            out, changed = [], False
            for inst in blk.instructions:
                if type(inst).__name__ == "InstISA" and inst.isa_opcode == 176:
                    m = _re.search(r"range_first=(\d+) range_last=(\d+)", inst.concise())
                    first, last = int(m.group(1)), int(m.group(2))
                    for semid in range(first, last + 1):
                        _ws_ctr[0] += 1
                        nop = mybir.InstNoOp(name=f"I-rc-{_ws_ctr[0]}", ins=[], outs=[])
                        nop.engine = inst.engine
                        nop.sync_info = bass_rust.SyncInfo(
                            on_wait=[],
                            on_update=[
                                bass_rust.SyncUpdate(
                                    sync_type="semaphore",
                                    id=semid,
                                    update_mode="sem-wr-imm",
                                    update_value=0,
                                )
                            ],
                        )
                        out.append(nop)
                    changed = True
                    continue
                out.append(inst)
            if changed:
                blk.instructions = out


def _split_excess_waits(nc, max_waits=1):
    """walrus on this image encodes at most one SyncWait per instruction;
    hoist excess waits onto same-engine NoOps placed just before."""
    for f in nc.m.functions:
        for blk in f.blocks:
            out, changed = [], False
            for inst in blk.instructions:
                si = inst.sync_info
                waits = list(si.on_wait) if si is not None else []
                if len(waits) > max_waits:
                    excess, keep = waits[:-max_waits], waits[-max_waits:]
                    for k in range(0, len(excess), max_waits):
                        _ws_ctr[0] += 1
                        nop = mybir.InstNoOp(name=f"I-ws-{_ws_ctr[0]}", ins=[], outs=[])
                        nop.engine = inst.engine
                        nop.sync_info = bass_rust.SyncInfo(
                            on_wait=excess[k : k + max_waits], on_update=[]
                        )
                        out.append(nop)
                    inst.sync_info = bass_rust.SyncInfo(
                        on_wait=keep, on_update=list(si.on_update)
                    )
                    changed = True
                out.append(inst)
            if changed:
                blk.instructions = out


def prep_inputs(inputs):
    """Host-side sharding/layout prep. Returns per-core input maps."""
    x = np.asarray(inputs["x"], np.float32)
    rope = np.asarray(inputs["rope_cache"], np.float32)
    wq = np.asarray(inputs["wq_w"], np.float32)
    bq = np.asarray(inputs["wq_b"], np.float32)
    wk = np.asarray(inputs["wk_w"], np.float32)
    bk = np.asarray(inputs["wk_b"], np.float32)
    wv = np.asarray(inputs["wv_w"], np.float32)
    bv = np.asarray(inputs["wv_b"], np.float32)
    wo = np.asarray(inputs["wo_w"], np.float32)
    sinks = np.asarray(inputs["sinks"], np.float32)

    xT = np.ascontiguousarray(x[0].T).astype(BF)            # [DIM, S]
    cosT = rope[:, :HD].T                                   # [64, S]
    sinT = rope[:, HD:].T
    cos2 = np.ascontiguousarray(np.concatenate([cosT, cosT], 0)).astype(np.float32)
    # sin_rot indexed by SOURCE partition: source rows hd in [0,32) land at
    # out rows hd+32 with +sin[hd+32]; source rows hd in [32,64) land at
    # out rows hd-32 with -sin[hd-32]. Duplicated for both heads per tile.
    sr = np.concatenate([sinT[32:64], -sinT[0:32]], 0)      # [64, S]
    sin_rot2 = np.ascontiguousarray(np.concatenate([sr, sr], 0)).astype(np.float32)
    tri = np.triu(np.ones((128, 128), BF))                  # mask[p, j] = j >= p
    ident = np.eye(HD, dtype=np.float32)
    ones_col = np.ones((128, 64), BF)
    ones_f = np.ones((128, 64), np.float32)

    in_maps = []
    for c in range(NCORES):
        qs = slice(c * QH * HD, (c + 1) * QH * HD)          # 256 q rows
        ks = slice(c * HD, (c + 1) * HD)                    # 64 kv rows
        # wproj columns: [q 256 | k 64 | v 64] = 384
        wproj = np.concatenate([wq[qs].T, wk[ks].T, wv[ks].T], axis=1)
        bcol = np.zeros((128, 3), np.float32)
        bcol[:, 0] = bq[qs][0:128]
        bcol[:, 1] = bq[qs][128:256]
        bcol[0:64, 2] = bk[ks]
        bcol[64:128, 2] = bv[ks]
        woT = np.ascontiguousarray(wo[:, qs].T).astype(BF)  # [256, DIM]
        esc = np.zeros((128, 1), np.float32)
        for h in range(QH):
            esc[32 * h, 0] = np.exp(sinks[c * QH + h])
        in_maps.append(
            {
                "xT": xT,
                "wproj": np.ascontiguousarray(wproj).astype(BF),
                "bproj": bcol,
                "cos2": cos2,
                "sinr2": sin_rot2,
                "woT": woT,
                "esinkc": esc,
                "tri": tri,
                "identf": ident,
                "onesb": ones_col,
                "onesf": ones_f,
            }
        )
    return in_maps


def build_nc(split_waits=True):
    nc = bass.Bass("TRN2", target_bir_lowering=False, debug=False, num_devices=NCORES)
    xT = nc.dram_tensor("xT", [DIM, S], BF16, kind="ExternalInput").ap()
    wproj = nc.dram_tensor("wproj", [DIM, 384], BF16, kind="ExternalInput").ap()
    bproj = nc.dram_tensor("bproj", [128, 3], F32, kind="ExternalInput").ap()
    cos2 = nc.dram_tensor("cos2", [128, S], F32, kind="ExternalInput").ap()
    sinr2 = nc.dram_tensor("sinr2", [128, S], F32, kind="ExternalInput").ap()
    woT = nc.dram_tensor("woT", [2 * 128, DIM], BF16, kind="ExternalInput").ap()
    esinkc = nc.dram_tensor("esinkc", [128, 1], F32, kind="ExternalInput").ap()
    tri = nc.dram_tensor("tri", [128, 128], BF16, kind="ExternalInput").ap()
    identf = nc.dram_tensor("identf", [HD, HD], F32, kind="ExternalInput").ap()
    onesb = nc.dram_tensor("onesb", [128, 64], BF16, kind="ExternalInput").ap()
    onesf = nc.dram_tensor("onesf", [128, 64], F32, kind="ExternalInput").ap()
    out = nc.dram_tensor("out", [S, DIM], BF16, kind="ExternalOutput").ap()

    with tile.TileContext(nc) as tc:
        with (
            tc.tile_pool(name="persist", bufs=1) as P,
            tc.tile_pool(name="ps_s", bufs=2, space="PSUM") as PS_S,
            tc.tile_pool(name="ps_p", bufs=1, space="PSUM") as PS_P,
            tc.tile_pool(name="ps_o", bufs=2, space="PSUM") as PS_O,
            tc.tile_pool(name="ps_d", bufs=1, space="PSUM") as PS_D,
            tc.tile_pool(name="tmp", bufs=2) as TMP,
            tc.tile_pool(name="ptp", bufs=6) as PT,
            tc.tile_pool(name="evp", bufs=4) as EV,
            tc.tile_pool(name="rnp", bufs=2) as RN,
        ):
            # ---- persistent tiles ----
            esink_t = P.tile([128, 1], F32, tag="esink")
            tri_t = P.tile([128, 128], BF16, tag="tri")
            wo_t = [P.tile([128, DIM], BF16, name=f"wo{i}", tag=f"wo{i}") for i in range(2)]
            qp = [P.tile([128, S], BF16, name=f"qp{i}", tag=f"qp{i}") for i in range(2)]
            kT2 = P.tile([128, S], BF16, tag="kT2")
            vTf = P.tile([64, S], F32, tag="vTf")
            vext = P.tile([128, NST * 64], BF16, tag="vext")
            outstk = [P.tile([128, S], BF16, name=f"os{i}", tag=f"os{i}") for i in range(2)]
            id_t = P.tile([HD, HD], F32, tag="idp")
            onesb_t = P.tile([128, 64], BF16, tag="onesb_t")
            onesf_t = P.tile([128, 64], F32, tag="onesf_t")
            bcol_t = P.tile([128, 3], F32, tag="bcol")
            cos_t = P.tile([128, S], F32, tag="cos")
            sinr_t = P.tile([128, S], F32, tag="sinr")
            scr = P.tile([1, 16], F32, tag="scr")
            x_t, w_t = [], []
            for dc in range(NDC):
                wt = P.tile([128, 384], BF16, name=f"w{dc}", tag=f"w{dc}")
                nc.gpsimd.dma_start(wt[:], wproj[dc * 128 : (dc + 1) * 128, :])
                w_t.append(wt)
            big = [P.tile([128, S], BF16, name=f"x{dc}", tag=f"x{dc}") for dc in range(NDC)]
            x_t = big
            for dc in range(NDC):
                nc.sync.dma_start(x_t[dc][:], xT[dc * 128 : (dc + 1) * 128, :])
            nc.gpsimd.dma_start(bcol_t[:], bproj[:])
            nc.gpsimd.dma_start(cos_t[:], cos2[:])
            nc.gpsimd.dma_start(sinr_t[:], sinr2[:])
            nc.gpsimd.dma_start(id_t[:], identf[:])
            nc.gpsimd.dma_start(onesb_t[:], onesb[:])
            nc.gpsimd.dma_start(onesf_t[:], onesf[:])
            nc.gpsimd.dma_start(esink_t[:], esinkc[:])
            nc.gpsimd.dma_start(tri_t[:], tri[:])
            for i in range(2):
                nc.gpsimd.dma_start(wo_t[i][:], woT[i * 128 : (i + 1) * 128, :])
            # pull the ACT Exp/Ln table load off the critical path
            nc.scalar.activation(scr[0:1, 0:3], bcol_t[0:1, 0:3], AF.Exp)
            nc.scalar.activation(scr[0:1, 0:3], scr[0:1, 0:3], AF.Ln)

            # ---------------- helpers (emit instructions) ----------------

            def rope_q(i, pp, ss):
                """Evict q tile i (heads 2i,2i+1) from psum pp with RoPE."""
                t1 = TMP.tile([128, SBLK], BF16, name="t1", tag="t1")
                nc.vector.scalar_tensor_tensor(
                    t1[:], pp, bcol_t[:, i : i + 1], cos_t[:, ss],
                    op0=OP.add, op1=OP.mult,
                )
                t2 = TMP.tile([128, SBLK], BF16, name="t2", tag="t2")
                for g in range(4):
                    d0 = 32 * g
                    s0 = 32 * g + 32 if g % 2 == 0 else 32 * g - 32
                    nc.vector.scalar_tensor_tensor(
                        t2[d0 : d0 + 32, :],
                        pp[s0 : s0 + 32, :],
                        bcol_t[s0 : s0 + 32, i : i + 1],
                        sinr_t[s0 : s0 + 32, ss],
                        op0=OP.add, op1=OP.mult,
                    )
                nc.vector.tensor_tensor(qp[i][:, ss], t1[:], t2[:], op=OP.add)

            def rope_kv(pp, ss):
                tk1 = TMP.tile([64, SBLK], BF16, name="tk1", tag="tk1")
                nc.vector.scalar_tensor_tensor(
                    tk1[:], pp[0:64, :], bcol_t[0:64, 2:3], cos_t[0:64, ss],
                    op0=OP.add, op1=OP.mult,
                )
                tk2 = TMP.tile([64, SBLK], BF16, name="tk2", tag="tk2")
                nc.vector.scalar_tensor_tensor(
                    tk2[0:32, :], pp[32:64, :], bcol_t[32:64, 2:3],
                    sinr_t[32:64, ss], op0=OP.add, op1=OP.mult,
                )
                nc.vector.scalar_tensor_tensor(
                    tk2[32:64, :], pp[0:32, :], bcol_t[0:32, 2:3],
                    sinr_t[0:32, ss], op0=OP.add, op1=OP.mult,
                )
                nc.vector.tensor_tensor(kT2[0:64, ss], tk1[:], tk2[:], op=OP.add)
                nc.vector.tensor_copy(kT2[64:128, ss], kT2[0:64, ss])
                # v rows with bias, fp32 (transposed later on PE)
                nc.vector.tensor_scalar_add(vTf[:, ss], pp[64:128, :], bcol_t[64:128, 2:3])

            def transp_round(r):
                """Transpose this round's 4 v tiles into vext via the S ring."""
                tp = PS_S.tile([128, 1024], F32, name="tp", tag="s")
                for j in range(4):
                    t = 4 * r + j
                    nc.tensor.transpose(
                        tp[:, j * 64 : (j + 1) * 64],
                        vTf[:, t * 128 : (t + 1) * 128],
                        id_t[:],
                    )
                    nc.vector.tensor_copy(
                        vext[:, t * 64 : (t + 1) * 64], tp[:, j * 64 : (j + 1) * 64]
                    )

            pso_cur = [None]
            den_cur = [None]

            def attn_iter(b, t):
                """One sk-tile iteration of attention block b."""
                off = 128 * (t - 4 * b) if t >= 4 * b else 0
                n0 = b * SBLK + off
                ptts = []
                for pi in range(2):
                    sbt = PS_S.tile([128, 1024], F32, name="sbt", tag="s")
                    # lane0 at [off:512], lane1 packed at [512:1024-off] so the
                    # exp input region is contiguous (no stale-data gap)
                    for lane, (c0, c1) in enumerate([(off, 512), (512, 1024 - off)]):
                        nc.tensor.matmul(
                            sbt[:, c0:c1],
                            kT2[64 * lane : 64 * lane + 64, t * 128 : (t + 1) * 128],
                            qp[pi][64 * lane : 64 * lane + 64, n0 : (b + 1) * SBLK],
                            start=True, stop=True,
                            tile_position=(64 * lane, 0),
                        )
                    ptt = PT.tile([128, 1024], BF16, name="ptt", tag="pt")
                    nc.scalar.activation(
                        ptt[:, off : 1024 - off], sbt[:, off : 1024 - off],
                        AF.Exp, scale=SCALE,
                    )
                    if t >= 4 * b:
                        for c in (off, 512):
                            nc.vector.tensor_tensor(
                                ptt[:, c : c + 128], ptt[:, c : c + 128],
                                tri_t[:], op=OP.mult,
                            )
                    ptts.append(ptt)
                if t == 0:
                    pso_cur[0] = [
                        PS_O.tile([128, SBLK], F32, name=f"pso{pi}", tag="o")
                        for pi in range(2)
                    ]
                    den_cur[0] = PS_D.tile([128, SBLK], F32, name="den", tag="d")
                pso, den = pso_cur[0], den_cur[0]
                vx = vext[:, t * 64 : (t + 1) * 64]
                first = t == 0
                last = t == 4 * b + 3
                for pi in range(2):
                    nc.tensor.matmul(
                        pso[pi][0:64, off:SBLK],
                        vx, ptts[pi][:, off:512],
                        start=first, stop=last, tile_position=(0, 0),
                    )
                    # skip_group_check: the sim's zero-region bookkeeping
                    # mis-translates partition-offset APs (aliases partitions
                    # 8..71); flags here mirror the checked even-half matmul
                    nc.tensor.matmul(
                        pso[pi][64:128, off:SBLK],
                        vx, ptts[pi][:, 512 : 1024 - off],
                        start=first, stop=last, tile_position=(0, 64),
                        skip_group_check=True,
                    )
                for h in range(QH):
                    pi, odd = h // 2, h % 2
                    # M=32 (same cycles as M=1) so the whole den bank is
                    # initialized for the full-tile renorm read later
                    nc.tensor.matmul(
                        den[32 * h : 32 * h + 32, off:SBLK],
                        onesb_t[:, 0:32],
                        ptts[pi][:, 512 * odd + off * (1 - odd) : 512 + 512 * odd - off * odd],
                        start=first, stop=last, tile_position=(0, 32 * h),
                        skip_group_check=(h > 0),
                    )

            def renorm_tail(b):
                """r = sumexp + e^sink; outstk = pso / r (broadcast via K=1 mm)."""
                bs = slice(b * SBLK, (b + 1) * SBLK)
                pso, den = pso_cur[0], den_cur[0]
                rowb = RN.tile([128, SBLK], F32, name="rowb", tag="rowb")
                nc.vector.tensor_scalar_add(rowb[:], den[:], esink_t[:])
                lnr = RN.tile([128, SBLK], F32, name="lnr", tag="lnr")
                nc.scalar.activation(lnr[:], rowb[:], AF.Ln)
                rinv = RN.tile([128, SBLK], F32, name="rinv", tag="rinv")
                nc.scalar.activation(rinv[:], lnr[:], AF.Exp, scale=-1.0)
                rbt = PS_S.tile([128, 1024], F32, name="rbt", tag="s")
                for h in range(QH):
                    pi, odd = h // 2, h % 2
                    nc.tensor.matmul(
                        rbt[64 * odd : 64 * odd + 64, 512 * pi : 512 * pi + 512],
                        onesf_t[32 * h : 32 * h + 1, :], rinv[32 * h : 32 * h + 1, :],
                        start=True, stop=True, tile_position=(32 * h, 64 * odd),
                    )
                rbs = RN.tile([128, 1024], F32, name="rbs", tag="rbs")
                nc.vector.tensor_copy(rbs[:], rbt[:])
                for pi in range(2):
                    nc.vector.tensor_tensor(
                        outstk[pi][:, bs], pso[pi][:],
                        rbs[:, 512 * pi : 512 * pi + 512], op=OP.mult,
                    )

            def outproj_tile(st, dpair):
                """Project sq tile st for output column pair dpair (2x512)."""
                psf = PS_S.tile([128, 1024], F32, name="psf", tag="s")
                for half in range(2):
                    db = 2 * dpair + half
                    ds = slice(db * SBLK, (db + 1) * SBLK)
                    nc.tensor.matmul(
                        psf[:, 512 * half : 512 * half + 512],
                        outstk[0][:, st * 128 : (st + 1) * 128], wo_t[0][:, ds],
                        start=True, stop=False,
                    )
                    nc.tensor.matmul(
                        psf[:, 512 * half : 512 * half + 512],
                        outstk[1][:, st * 128 : (st + 1) * 128], wo_t[1][:, ds],
                        start=False, stop=True,
                    )
                    ot = EV.tile([128, SBLK], BF16, name="ot", tag="ev")
                    nc.vector.tensor_copy(ot[:], psf[:, 512 * half : 512 * half + 512])
                    nc.sync.dma_start(out[st * 128 : (st + 1) * 128, ds], ot[:])

            def outproj_block(b):
                for st in range(4 * b, 4 * b + 4):
                    for dpair in range(2):
                        outproj_tile(st, dpair)

            def proj_group(g, ss, pp):
                c0 = 128 * g if g < 2 else 256
                c1 = c0 + 128
                for dc in range(NDC):
                    nc.tensor.matmul(
                        pp, w_t[dc][:, c0:c1], x_t[dc][:, ss],
                        start=(dc == 0), stop=(dc == NDC - 1),
                    )

            # ---------------- round 0: projection of block 0 ----------------
            ss0 = slice(0, SBLK)
            s_q = PS_S.tile([128, 1024], F32, name="s_q", tag="s")
            s_k = PS_S.tile([128, 1024], F32, name="s_k", tag="s")
            for dc in range(NDC):
                nc.tensor.matmul(
                    s_q[:, 0:512], w_t[dc][:, 0:128], x_t[dc][:, ss0],
                    start=(dc == 0), stop=(dc == NDC - 1),
                )
                nc.tensor.matmul(
                    s_q[:, 512:1024], w_t[dc][:, 128:256], x_t[dc][:, ss0],
                    start=(dc == 0), stop=(dc == NDC - 1),
                )
                nc.tensor.matmul(
                    s_k[:, 0:512], w_t[dc][:, 256:384], x_t[dc][:, ss0],
                    start=(dc == 0), stop=(dc == NDC - 1),
                )
            rope_q(0, s_q[:, 0:512], ss0)
            rope_q(1, s_q[:, 512:1024], ss0)
            rope_kv(s_k[:, 0:512], ss0)
            transp_round(0)

            # ---------------- rounds 1..3: proj r + attn r-1 ----------------
            # iter split points per round: after q0 group, after q1 group
            for r in range(1, NSB):
                b = r - 1
                nt = 4 * b + 4
                ssr = slice(r * SBLK, (r + 1) * SBLK)
                k1 = max(1, nt // 3)
                k2 = max(2, (2 * nt) // 3)
                pp_q0 = PS_P.tile([128, SBLK], F32, name="pp", tag="p")
                proj_group(0, ssr, pp_q0[:])
                rope_q(0, pp_q0[:], ssr)
                for t in range(0, k1):
                    attn_iter(b, t)
                pp_q1 = PS_P.tile([128, SBLK], F32, name="pp", tag="p")
                proj_group(1, ssr, pp_q1[:])
                rope_q(1, pp_q1[:], ssr)
                for t in range(k1, k2):
                    attn_iter(b, t)
                pp_kv = PS_P.tile([128, SBLK], F32, name="pp", tag="p")
                proj_group(2, ssr, pp_kv[:])
                rope_kv(pp_kv[:], ssr)
                for t in range(k2, nt):
                    attn_iter(b, t)
                renorm_tail(b)
                transp_round(r)
                if r == 3:
                    # prefetch early block-3 iters while outproj(2) runs
                    for t in range(0, 6):
                        attn_iter(3, t)
                outproj_block(b)

            # ---------------- round 4: finish block 3 ----------------
            for t in range(6, 16):
                attn_iter(3, t)
            renorm_tail(3)
            outproj_block(3)

    _fix_range_clears(nc)
    if split_waits:
        _split_excess_waits(nc)
    return nc


_nc_cache = [None]


def kernel(**inputs):
    in_maps = prep_inputs(inputs)
    if _nc_cache[0] is None:
        _nc_cache[0] = build_nc()
    nc = _nc_cache[0]
    res = run_bass_kernel_spmd(nc, in_maps, list(range(NCORES)))
    acc = res.results[0]["out"].astype(np.float32)
    for i in range(1, NCORES):
        acc = acc + res.results[i]["out"].astype(np.float32)
    acc = acc + np.asarray(inputs["wo_b"], np.float32).reshape(1, DIM)
    return acc.reshape(B, S, DIM)


# revision 14
# speedup vs baseline: 1.2194x; 1.0361x over previous
"""Trainium2 Bass kernel for nn_Attention_4037269258732 (GQA attention with
RoPE, causal mask, and per-head sink-logit LSE renormalization).

Problem:  B=1, S=2048, DIM=2048, H=32 q-heads, KVH=8 kv-heads, HD=64.
          out = Wo @ attn(RoPE(Wq x), RoPE(Wk x), Wv x) + bo, causal,
          with out rows scaled by r = sumexp/(sumexp + e^sink).

Sharding (8 cores, tensor-parallel over heads):
  core c owns q-heads [4c, 4c+4), kv-head c, the matching rows of
  wq/wk/wv, wo's input-dim slice [256c, 256c+256), and sinks[4c:4c+4].
  Each core computes a full-shape [S, DIM] bf16 partial of the output
  projection (no bias); the host sums the 8 partials and adds wo_b.

v2 layout: merged software pipeline of 5 rounds.  Round r interleaves, in
PE program order, the QKV projection of sequence block r with the
attention of block r-1, so the PE never idles long enough to trip the
HAM clock gate and the ACT-engine exp evictions overlap matmuls.

  - scores: per head-pair, two row-packed K=64 matmuls into one
    [128,1024] fp32 PSUM tile (2 banks); a single wide ACT Exp evicts
    both heads at once (amortizes the 352-cycle ACT fixed cost).
  - PV: col-tiled M=64 pairs (tile_position (0,0)/(0,64)) accumulate two
    heads into one PSUM bank concurrently; per-head softmax denominators
    via 4 concurrent M=1 matmuls (cols 0/32/64/96) into one bank.
  - PSUM budget (8 banks): S-ring 2x[128,1024] (4) shared by scores,
    V-transposes, renorm broadcasts and outproj accumulators; proj 1;
    pso 2; denom 1.
"""

import numpy as np
import ml_dtypes

import bass_rust
import concourse.bass as bass
import concourse.tile as tile
from concourse import mybir
from concourse.bass_utils import run_bass_kernel_spmd

F32 = mybir.dt.float32
BF16 = mybir.dt.bfloat16
AF = mybir.ActivationFunctionType
OP = mybir.AluOpType
BF = ml_dtypes.bfloat16

B, S, DIM = 1, 2048, 2048
H, KVH, HD = 32, 8, 64
NCORES = 8
QH = H // NCORES          # 4 q heads per core
SBLK = 512                # sq block size
NSB = S // SBLK           # 4
NDC = DIM // 128          # 16 contraction chunks
NST = S // 128            # 16 sk tiles
SCALE = 1.0 / 8.0         # 1/sqrt(HD)

_ws_ctr = [0]


def _fix_range_clears(nc):
    """walrus here rejects the EVENT_SEMAPHORE_RANGE_CLEAR ISA struct
    ("ISA wrong length"); replace with per-sem write-0 NoOps."""
    import re as _re
    for f in nc.m.functions:
        for blk in f.blocks:
            out, changed = [], False
            for inst in blk.instructions:
                if type(inst).__name__ == "InstISA" and inst.isa_opcode == 176:
                    m = _re.search(r"range_first=(\d+) range_last=(\d+)", inst.concise())
                    first, last = int(m.group(1)), int(m.group(2))
                    for semid in range(first, last + 1):
                        _ws_ctr[0] += 1
                        nop = mybir.InstNoOp(name=f"I-rc-{_ws_ctr[0]}", ins=[], outs=[])
                        nop.engine = inst.engine
                        nop.sync_info = bass_rust.SyncInfo(
                            on_wait=[],
                            on_update=[
                                bass_rust.SyncUpdate(
                                    sync_type="semaphore",
                                    id=semid,
                                    update_mode="sem-wr-imm",
                                    update_value=0,
                                )
                            ],
                        )
                        out.append(nop)
                    changed = True
                    continue
                out.append(inst)
            if changed:
                blk.instructions = out


def _split_excess_waits(nc, max_waits=1):
    """walrus on this image encodes at most one SyncWait per instruction;
    hoist excess waits onto same-engine NoOps placed just before."""
    for f in nc.m.functions:
        for blk in f.blocks:
            out, changed = [], False
            for inst in blk.instructions:
                si = inst.sync_info
                waits = list(si.on_wait) if si is not None else []
                if len(waits) > max_waits:
                    excess, keep = waits[:-max_waits], waits[-max_waits:]
                    for k in range(0, len(excess), max_waits):
                        _ws_ctr[0] += 1
                        nop = mybir.InstNoOp(name=f"I-ws-{_ws_ctr[0]}", ins=[], outs=[])
                        nop.engine = inst.engine
                        nop.sync_info = bass_rust.SyncInfo(
                            on_wait=excess[k : k + max_waits], on_update=[]
                        )
                        out.append(nop)
                    inst.sync_info = bass_rust.SyncInfo(
                        on_wait=keep, on_update=list(si.on_update)
                    )
                    changed = True
                out.append(inst)
            if changed:
                blk.instructions = out


def prep_inputs(inputs):
    """Host-side sharding/layout prep. Returns per-core input maps."""
    x = np.asarray(inputs["x"], np.float32)
    rope = np.asarray(inputs["rope_cache"], np.float32)
    wq = np.asarray(inputs["wq_w"], np.float32)
    bq = np.asarray(inputs["wq_b"], np.float32)
    wk = np.asarray(inputs["wk_w"], np.float32)
    bk = np.asarray(inputs["wk_b"], np.float32)
    wv = np.asarray(inputs["wv_w"], np.float32)
    bv = np.asarray(inputs["wv_b"], np.float32)
    wo = np.asarray(inputs["wo_w"], np.float32)
    sinks = np.asarray(inputs["sinks"], np.float32)

    xT = np.ascontiguousarray(x[0].T).astype(BF)            # [DIM, S]
    cosT = rope[:, :HD].T                                   # [64, S]
    sinT = rope[:, HD:].T
    cos2 = np.ascontiguousarray(np.concatenate([cosT, cosT], 0)).astype(BF)
    # sin_rot indexed by SOURCE partition: source rows hd in [0,32) land at
    # out rows hd+32 with +sin[hd+32]; source rows hd in [32,64) land at
    # out rows hd-32 with -sin[hd-32]. Duplicated for both heads per tile.
    sr = np.concatenate([sinT[32:64], -sinT[0:32]], 0)      # [64, S]
    sin_rot2 = np.ascontiguousarray(np.concatenate([sr, sr], 0)).astype(BF)
    tri = np.triu(np.ones((128, 128), BF))                  # mask[p, j] = j >= p
    ident = np.eye(HD, dtype=np.float32)
    ones_col = np.ones((128, 64), BF)
    ones_f = np.ones((128, 64), np.float32)

    in_maps = []
    for c in range(NCORES):
        qs = slice(c * QH * HD, (c + 1) * QH * HD)          # 256 q rows
        ks = slice(c * HD, (c + 1) * HD)                    # 64 kv rows
        # wproj columns: [q 256 | k 64 | v 64] = 384
        wproj = np.concatenate([wq[qs].T, wk[ks].T, wv[ks].T], axis=1)
        bcol = np.zeros((128, 3), np.float32)
        bcol[:, 0] = bq[qs][0:128]
        bcol[:, 1] = bq[qs][128:256]
        bcol[0:64, 2] = bk[ks]
        bcol[64:128, 2] = bv[ks]
        woT = np.ascontiguousarray(wo[:, qs].T).astype(BF)  # [256, DIM]
        esc = np.zeros((128, 1), np.float32)
        for h in range(QH):
            esc[32 * h, 0] = np.exp(sinks[c * QH + h])
        in_maps.append(
            {
                "xT": xT,
                "wproj": np.ascontiguousarray(wproj).astype(BF),
                "bproj": bcol,
                "cos2": cos2,
                "sinr2": sin_rot2,
                "woT": woT,
                "esinkc": esc,
                "tri": tri,
                "identf": ident,
                "onesb": ones_col,
                "onesf": ones_f,
            }
        )
    return in_maps


def build_nc(split_waits=True):
    nc = bass.Bass("TRN2", target_bir_lowering=False, debug=False, num_devices=NCORES)
    xT = nc.dram_tensor("xT", [DIM, S], BF16, kind="ExternalInput").ap()
    wproj = nc.dram_tensor("wproj", [DIM, 384], BF16, kind="ExternalInput").ap()
    bproj = nc.dram_tensor("bproj", [128, 3], F32, kind="ExternalInput").ap()
    cos2 = nc.dram_tensor("cos2", [128, S], BF16, kind="ExternalInput").ap()
    sinr2 = nc.dram_tensor("sinr2", [128, S], BF16, kind="ExternalInput").ap()
    woT = nc.dram_tensor("woT", [2 * 128, DIM], BF16, kind="ExternalInput").ap()
    esinkc = nc.dram_tensor("esinkc", [128, 1], F32, kind="ExternalInput").ap()
    tri = nc.dram_tensor("tri", [128, 128], BF16, kind="ExternalInput").ap()
    identf = nc.dram_tensor("identf", [HD, HD], F32, kind="ExternalInput").ap()
    onesb = nc.dram_tensor("onesb", [128, 64], BF16, kind="ExternalInput").ap()
    onesf = nc.dram_tensor("onesf", [128, 64], F32, kind="ExternalInput").ap()
    out = nc.dram_tensor("out", [S, DIM], BF16, kind="ExternalOutput").ap()

    with tile.TileContext(nc) as tc:
        with (
            tc.tile_pool(name="persist", bufs=1) as P,
            tc.tile_pool(name="ps_s", bufs=2, space="PSUM") as PS_S,
            tc.tile_pool(name="ps_p", bufs=1, space="PSUM") as PS_P,
            tc.tile_pool(name="ps_o", bufs=2, space="PSUM") as PS_O,
            tc.tile_pool(name="ps_d", bufs=1, space="PSUM") as PS_D,
            tc.tile_pool(name="tmp", bufs=2) as TMP,
            tc.tile_pool(name="ptp", bufs=6) as PT,
            tc.tile_pool(name="evp", bufs=4) as EV,
            tc.tile_pool(name="rnp", bufs=2) as RN,
        ):
            # ---- persistent tiles ----
            esink_t = P.tile([128, 1], F32, tag="esink")
            tri_t = P.tile([128, 128], BF16, tag="tri")
            wo_t = [P.tile([128, DIM], BF16, name=f"wo{i}", tag=f"wo{i}") for i in range(2)]
            qp = [P.tile([128, S], BF16, name=f"qp{i}", tag=f"qp{i}") for i in range(2)]
            kT2 = P.tile([128, S], BF16, tag="kT2")
            vTf = P.tile([64, S], F32, tag="vTf")
            vext = P.tile([128, NST * 64], BF16, tag="vext")
            outstk = [P.tile([128, S], BF16, name=f"os{i}", tag=f"os{i}") for i in range(2)]
            id_t = P.tile([HD, HD], F32, tag="idp")
            onesb_t = P.tile([128, 64], BF16, tag="onesb_t")
            onesf_t = P.tile([128, 64], F32, tag="onesf_t")
            bcol_t = P.tile([128, 3], F32, tag="bcol")
            cos_t = P.tile([128, S], BF16, tag="cos")
            sinr_t = P.tile([128, S], BF16, tag="sinr")
            scr = P.tile([1, 16], F32, tag="scr")
            x_t, w_t = [], []
            for dc in range(NDC):
                wt = P.tile([128, 384], BF16, name=f"w{dc}", tag=f"w{dc}")
                nc.gpsimd.dma_start(wt[:], wproj[dc * 128 : (dc + 1) * 128, :])
                w_t.append(wt)
            big = [P.tile([128, S], BF16, name=f"x{dc}", tag=f"x{dc}") for dc in range(NDC)]
            x_t = big
            for dc in range(NDC):
                nc.sync.dma_start(x_t[dc][:], xT[dc * 128 : (dc + 1) * 128, :])
            nc.gpsimd.dma_start(bcol_t[:], bproj[:])
            nc.gpsimd.dma_start(cos_t[:], cos2[:])
            nc.gpsimd.dma_start(sinr_t[:], sinr2[:])
            nc.gpsimd.dma_start(id_t[:], identf[:])
            nc.gpsimd.dma_start(onesb_t[:], onesb[:])
            nc.gpsimd.dma_start(onesf_t[:], onesf[:])
            nc.gpsimd.dma_start(esink_t[:], esinkc[:])
            nc.gpsimd.dma_start(tri_t[:], tri[:])
            for i in range(2):
                nc.gpsimd.dma_start(wo_t[i][:], woT[i * 128 : (i + 1) * 128, :])
            # pull the ACT Exp/Ln table load off the critical path
            nc.scalar.activation(scr[0:1, 0:3], bcol_t[0:1, 0:3], AF.Exp)
            nc.scalar.activation(scr[0:1, 0:3], scr[0:1, 0:3], AF.Ln)

            # ---------------- helpers (emit instructions) ----------------

            def rope_q(i, pp, ss):
                """Evict q tile i from psum pp (one fast read), then RoPE in
                bf16 on SBUF (2x DVE mode); frees the proj psum bank early."""
                u = TMP.tile([128, SBLK], BF16, name="u", tag="u")
                nc.vector.tensor_scalar_add(u[:], pp, bcol_t[:, i : i + 1])
                t1 = TMP.tile([128, SBLK], BF16, name="t1", tag="t1")
                nc.vector.tensor_tensor(t1[:], u[:], cos_t[:, ss], op=OP.mult)
                t2 = TMP.tile([128, SBLK], BF16, name="t2", tag="t2")
                for g in range(4):
                    d0 = 32 * g
                    s0 = d0 + 32 if g % 2 == 0 else d0 - 32
                    nc.vector.tensor_tensor(
                        t2[d0 : d0 + 32, :], u[s0 : s0 + 32, :],
                        sinr_t[s0 : s0 + 32, ss], op=OP.mult,
                    )
                nc.vector.tensor_tensor(qp[i][:, ss], t1[:], t2[:], op=OP.add)

            def rope_kv(pp, ss):
                uk = TMP.tile([64, SBLK], BF16, name="uk", tag="uk")
                nc.vector.tensor_scalar_add(uk[:], pp[0:64, :], bcol_t[0:64, 2:3])
                # v rows with bias, fp32 (transposed later on PE)
                nc.vector.tensor_scalar_add(vTf[:, ss], pp[64:128, :], bcol_t[64:128, 2:3])
                tk1 = TMP.tile([64, SBLK], BF16, name="tk1", tag="tk1")
                nc.vector.tensor_tensor(tk1[:], uk[:], cos_t[0:64, ss], op=OP.mult)
                tk2 = TMP.tile([64, SBLK], BF16, name="tk2", tag="tk2")
                nc.vector.tensor_tensor(
                    tk2[0:32, :], uk[32:64, :], sinr_t[32:64, ss], op=OP.mult
                )
                nc.vector.tensor_tensor(
                    tk2[32:64, :], uk[0:32, :], sinr_t[0:32, ss], op=OP.mult
                )
                nc.vector.tensor_tensor(kT2[0:64, ss], tk1[:], tk2[:], op=OP.add)
                nc.vector.tensor_copy(kT2[64:128, ss], kT2[0:64, ss])

            def transp_round(r):
                """Transpose this round's 4 v tiles into vext via the S ring."""
                tp = PS_S.tile([128, 1024], F32, name="tp", tag="s")
                for j in range(4):
                    t = 4 * r + j
                    nc.tensor.transpose(
                        tp[:, j * 64 : (j + 1) * 64],
                        vTf[:, t * 128 : (t + 1) * 128],
                        id_t[:],
                    )
                    nc.vector.tensor_copy(
                        vext[:, t * 64 : (t + 1) * 64], tp[:, j * 64 : (j + 1) * 64]
                    )

            pso_cur = [None]
            den_cur = [None]
            ptt_store = {}

            def s_iter(b, t):
                """Scores + exp for sk-tile t of block b (both head pairs)."""
                off = 128 * (t - 4 * b) if t >= 4 * b else 0
                n0 = b * SBLK + off
                ptts = []
                for pi in range(2):
                    sbt = PS_S.tile([128, 1024], F32, name="sbt", tag="s")
                    # lane0 at [off:512], lane1 packed at [512:1024-off] so the
                    # exp input region is contiguous (no stale-data gap)
                    for lane, (c0, c1) in enumerate([(off, 512), (512, 1024 - off)]):
                        nc.tensor.matmul(
                            sbt[:, c0:c1],
                            kT2[64 * lane : 64 * lane + 64, t * 128 : (t + 1) * 128],
                            qp[pi][64 * lane : 64 * lane + 64, n0 : (b + 1) * SBLK],
                            start=True, stop=True,
                            tile_position=(64 * lane, 0),
                        )
                    ptt = PT.tile([128, 1024], BF16, name="ptt", tag="pt")
                    nc.scalar.activation(
                        ptt[:, off : 1024 - off], sbt[:, off : 1024 - off],
                        AF.Exp, scale=SCALE,
                    )
                    if t >= 4 * b:
                        for c in (off, 512):
                            nc.vector.tensor_tensor(
                                ptt[:, c : c + 128], ptt[:, c : c + 128],
                                tri_t[:], op=OP.mult,
                            )
                    ptts.append(ptt)
                ptt_store[(b, t)] = ptts

            def pv_iter(b, t):
                """PV accumulation + denominators for sk-tile t of block b."""
                off = 128 * (t - 4 * b) if t >= 4 * b else 0
                ptts = ptt_store.pop((b, t))
                if t == 0:
                    pso_cur[0] = [
                        PS_O.tile([128, SBLK], F32, name=f"pso{pi}", tag="o")
                        for pi in range(2)
                    ]
                    den_cur[0] = PS_D.tile([128, SBLK], F32, name="den", tag="d")
                pso, den = pso_cur[0], den_cur[0]
                vx = vext[:, t * 64 : (t + 1) * 64]
                first = t == 0
                last = t == 4 * b + 3
                for pi in range(2):
                    nc.tensor.matmul(
                        pso[pi][0:64, off:SBLK],
                        vx, ptts[pi][:, off:512],
                        start=first, stop=last, tile_position=(0, 0),
                    )
                    # skip_group_check: the sim's zero-region bookkeeping
                    # mis-translates partition-offset APs (aliases partitions
                    # 8..71); flags here mirror the checked even-half matmul
                    nc.tensor.matmul(
                        pso[pi][64:128, off:SBLK],
                        vx, ptts[pi][:, 512 : 1024 - off],
                        start=first, stop=last, tile_position=(0, 64),
                        skip_group_check=True,
                    )
                for h in range(QH):
                    pi, odd = h // 2, h % 2
                    # M=32 (same cycles as M=1) so the whole den bank is
                    # initialized for the full-tile renorm read later
                    nc.tensor.matmul(
                        den[32 * h : 32 * h + 32, off:SBLK],
                        onesb_t[:, 0:32],
                        ptts[pi][:, 512 * odd + off * (1 - odd) : 512 + 512 * odd - off * odd],
                        start=first, stop=last, tile_position=(0, 32 * h),
                        skip_group_check=(h > 0),
                    )

            def renorm_tail(b):
                """r = sumexp + e^sink; outstk = pso / r (broadcast via K=1 mm)."""
                bs = slice(b * SBLK, (b + 1) * SBLK)
                pso, den = pso_cur[0], den_cur[0]
                rowb = RN.tile([128, SBLK], F32, name="rowb", tag="rowb")
                nc.vector.tensor_scalar_add(rowb[:], den[:], esink_t[:])
                lnr = RN.tile([128, SBLK], F32, name="lnr", tag="lnr")
                nc.scalar.activation(lnr[:], rowb[:], AF.Ln)
                rinv = RN.tile([128, SBLK], F32, name="rinv", tag="rinv")
                nc.scalar.activation(rinv[:], lnr[:], AF.Exp, scale=-1.0)
                rbt = PS_S.tile([128, 1024], F32, name="rbt", tag="s")
                for h in range(QH):
                    pi, odd = h // 2, h % 2
                    nc.tensor.matmul(
                        rbt[64 * odd : 64 * odd + 64, 512 * pi : 512 * pi + 512],
                        onesf_t[32 * h : 32 * h + 1, :], rinv[32 * h : 32 * h + 1, :],
                        start=True, stop=True, tile_position=(32 * h, 64 * odd),
                    )
                rbs = RN.tile([128, 1024], F32, name="rbs", tag="rbs")
                nc.vector.tensor_copy(rbs[:], rbt[:])
                for pi in range(2):
                    nc.vector.tensor_tensor(
                        outstk[pi][:, bs], pso[pi][:],
                        rbs[:, 512 * pi : 512 * pi + 512], op=OP.mult,
                    )

            def outproj_tile(st, dpair, split_cast=False):
                """Project sq tile st for output column pair dpair (2x512)."""
                psf = PS_S.tile([128, 1024], F32, name="psf", tag="s")
                for half in range(2):
                    db = 2 * dpair + half
                    ds = slice(db * SBLK, (db + 1) * SBLK)
                    nc.tensor.matmul(
                        psf[:, 512 * half : 512 * half + 512],
                        outstk[0][:, st * 128 : (st + 1) * 128], wo_t[0][:, ds],
                        start=True, stop=False,
                    )
                    nc.tensor.matmul(
                        psf[:, 512 * half : 512 * half + 512],
                        outstk[1][:, st * 128 : (st + 1) * 128], wo_t[1][:, ds],
                        start=False, stop=True,
                    )
                    ot = EV.tile([128, SBLK], BF16, name="ot", tag="ev")
                    if split_cast and half == 1:
                        nc.scalar.copy(ot[:], psf[:, 512 * half : 512 * half + 512])
                    else:
                        nc.vector.tensor_copy(ot[:], psf[:, 512 * half : 512 * half + 512])
                    nc.sync.dma_start(out[st * 128 : (st + 1) * 128, ds], ot[:])

            def proj_group(g, ss, pp):
                c0 = 128 * g if g < 2 else 256
                c1 = c0 + 128
                for dc in range(NDC):
                    nc.tensor.matmul(
                        pp, w_t[dc][:, c0:c1], x_t[dc][:, ss],
                        start=(dc == 0), stop=(dc == NDC - 1),
                    )

            # ---------------- round 0: projection of block 0 ----------------
            ss0 = slice(0, SBLK)
            s_q = PS_S.tile([128, 1024], F32, name="s_q", tag="s")
            s_k = PS_S.tile([128, 1024], F32, name="s_k", tag="s")
            for dc in range(NDC):
                nc.tensor.matmul(
                    s_q[:, 0:512], w_t[dc][:, 0:128], x_t[dc][:, ss0],
                    start=(dc == 0), stop=(dc == NDC - 1),
                )
                nc.tensor.matmul(
                    s_q[:, 512:1024], w_t[dc][:, 128:256], x_t[dc][:, ss0],
                    start=(dc == 0), stop=(dc == NDC - 1),
                )
                nc.tensor.matmul(
                    s_k[:, 0:512], w_t[dc][:, 256:384], x_t[dc][:, ss0],
                    start=(dc == 0), stop=(dc == NDC - 1),
                )
            rope_q(0, s_q[:, 0:512], ss0)
            rope_q(1, s_q[:, 512:1024], ss0)
            rope_kv(s_k[:, 0:512], ss0)
            transp_round(0)
            s_iter(0, 0)
            s_iter(0, 1)

            # ------- rounds 1..3: proj r + attn r-1, score-lead pipeline -----
            for r in range(1, NSB):
                b = r - 1
                nt = 4 * b + 4
                ssr = slice(r * SBLK, (r + 1) * SBLK)

                fillers = []

                def mk_proj(g, ssr=ssr):
                    def f():
                        pp = PS_P.tile([128, SBLK], F32, name="pp", tag="p")
                        proj_group(g, ssr, pp[:])
                        if g < 2:
                            rope_q(g, pp[:], ssr)
                        else:
                            rope_kv(pp[:], ssr)
                    return f

                for g in range(3):
                    fillers.append(mk_proj(g))
                if b >= 1:
                    for st in range(4 * (b - 1), 4 * (b - 1) + 4):
                        for dp in range(2):
                            fillers.append(
                                lambda st=st, dp=dp: outproj_tile(st, dp)
                            )
                fi = 0
                for i in range(nt):
                    if i + 2 < nt:
                        s_iter(b, i + 2)
                    pv_iter(b, i)
                    while fi < len(fillers) and (fi + 1) * nt <= (i + 1) * len(fillers):
                        fillers[fi]()
                        fi += 1
                while fi < len(fillers):
                    fillers[fi]()
                    fi += 1
                renorm_tail(b)
                transp_round(r)
                s_iter(r, 0)
                s_iter(r, 1)

            # ------- block 3 early iters, interleaved with outproj(2) -------
            ofill = [
                (lambda st=st, dp=dp: outproj_tile(st, dp))
                for st in range(8, 12) for dp in range(2)
            ]
            fi = 0
            for i in range(12):
                if i + 2 < 16:
                    s_iter(3, i + 2)
                pv_iter(3, i)
                while fi < len(ofill) and (fi + 1) * 12 <= (i + 1) * len(ofill):
                    ofill[fi]()
                    fi += 1
            while fi < len(ofill):
                ofill[fi]()
                fi += 1

            # ---------------- round 4: finish block 3 ----------------
            for i in range(12, 16):
                if i + 2 < 16:
                    s_iter(3, i + 2)
                pv_iter(3, i)
            renorm_tail(3)
            for st in range(12, 16):
                for dp in range(2):
                    outproj_tile(st, dp, split_cast=True)

    _fix_range_clears(nc)
    if split_waits:
        _split_excess_waits(nc)
    return nc


_nc_cache = [None]


def kernel(**inputs):
    in_maps = prep_inputs(inputs)
    if _nc_cache[0] is None:
        _nc_cache[0] = build_nc()
    nc = _nc_cache[0]
    res = run_bass_kernel_spmd(nc, in_maps, list(range(NCORES)))
    acc = res.results[0]["out"].astype(np.float32)
    for i in range(1, NCORES):
        acc = acc + res.results[i]["out"].astype(np.float32)
    acc = acc + np.asarray(inputs["wo_b"], np.float32).reshape(1, DIM)
    return acc.reshape(B, S, DIM)


# revision 17
# speedup vs baseline: 1.4455x; 1.1854x over previous
"""Trainium2 Bass kernel for nn_Attention_4037269258732 (GQA attention with
RoPE, causal mask, and per-head sink-logit LSE renormalization).

Problem:  B=1, S=2048, DIM=2048, H=32 q-heads, KVH=8 kv-heads, HD=64.
          out = Wo @ attn(RoPE(Wq x), RoPE(Wk x), Wv x) + bo, causal,
          with out rows scaled by r = sumexp/(sumexp + e^sink).

Sharding (8 cores, tensor-parallel over heads):
  core c owns q-heads [4c, 4c+4), kv-head c, the matching rows of
  wq/wk/wv, wo's input-dim slice [256c, 256c+256), and sinks[4c:4c+4].
  Each core computes a full-shape [S, DIM] bf16 partial of the output
  projection (no bias); the host sums the 8 partials and adds wo_b.

v2 layout: merged software pipeline of 5 rounds.  Round r interleaves, in
PE program order, the QKV projection of sequence block r with the
attention of block r-1, so the PE never idles long enough to trip the
HAM clock gate and the ACT-engine exp evictions overlap matmuls.

  - scores: per head-pair, two row-packed K=64 matmuls into one
    [128,1024] fp32 PSUM tile (2 banks); a single wide ACT Exp evicts
    both heads at once (amortizes the 352-cycle ACT fixed cost).
  - PV: col-tiled M=64 pairs (tile_position (0,0)/(0,64)) accumulate two
    heads into one PSUM bank concurrently; per-head softmax denominators
    via 4 concurrent M=1 matmuls (cols 0/32/64/96) into one bank.
  - PSUM budget (8 banks): S-ring 2x[128,1024] (4) shared by scores,
    V-transposes, renorm broadcasts and outproj accumulators; proj 1;
    pso 2; denom 1.
"""

import numpy as np
import ml_dtypes

import bass_rust
import concourse.bass as bass
import concourse.tile as tile
from concourse import mybir
from concourse.bass_utils import run_bass_kernel_spmd

F32 = mybir.dt.float32
BF16 = mybir.dt.bfloat16
AF = mybir.ActivationFunctionType
OP = mybir.AluOpType
BF = ml_dtypes.bfloat16

B, S, DIM = 1, 2048, 2048
H, KVH, HD = 32, 8, 64
NCORES = 8
QH = H // NCORES          # 4 q heads per core
SBLK = 512                # sq block size
NSB = S // SBLK           # 4
NDC = DIM // 128          # 16 contraction chunks
NST = S // 128            # 16 sk tiles
SCALE = 1.0 / 8.0         # 1/sqrt(HD)

_ws_ctr = [0]


def _fix_range_clears(nc):
    """walrus here rejects the EVENT_SEMAPHORE_RANGE_CLEAR ISA struct
    ("ISA wrong length"); replace with per-sem write-0 NoOps."""
    import re as _re
    for f in nc.m.functions:
        for blk in f.blocks:
            out, changed = [], False
            for inst in blk.instructions:
                if type(inst).__name__ == "InstISA" and inst.isa_opcode == 176:
                    m = _re.search(r"range_first=(\d+) range_last=(\d+)", inst.concise())
                    first, last = int(m.group(1)), int(m.group(2))
                    for semid in range(first, last + 1):
                        _ws_ctr[0] += 1
                        nop = mybir.InstNoOp(name=f"I-rc-{_ws_ctr[0]}", ins=[], outs=[])
                        nop.engine = inst.engine
                        nop.sync_info = bass_rust.SyncInfo(
                            on_wait=[],
                            on_update=[
                                bass_rust.SyncUpdate(
                                    sync_type="semaphore",
                                    id=semid,
                                    update_mode="sem-wr-imm",
                                    update_value=0,
                                )
                            ],
                        )
                        out.append(nop)
                    changed = True
                    continue
                out.append(inst)
            if changed:
                blk.instructions = out


def _split_excess_waits(nc, max_waits=1):
    """walrus on this image encodes at most one SyncWait per instruction;
    hoist excess waits onto same-engine NoOps placed just before."""
    for f in nc.m.functions:
        for blk in f.blocks:
            out, changed = [], False
            for inst in blk.instructions:
                si = inst.sync_info
                waits = list(si.on_wait) if si is not None else []
                if len(waits) > max_waits:
                    excess, keep = waits[:-max_waits], waits[-max_waits:]
                    for k in range(0, len(excess), max_waits):
                        _ws_ctr[0] += 1
                        nop = mybir.InstNoOp(name=f"I-ws-{_ws_ctr[0]}", ins=[], outs=[])
                        nop.engine = inst.engine
                        nop.sync_info = bass_rust.SyncInfo(
                            on_wait=excess[k : k + max_waits], on_update=[]
                        )
                        out.append(nop)
                    inst.sync_info = bass_rust.SyncInfo(
                        on_wait=keep, on_update=list(si.on_update)
                    )
                    changed = True
                out.append(inst)
            if changed:
                blk.instructions = out


def prep_inputs(inputs):
    """Host-side sharding/layout prep. Returns per-core input maps."""
    x = np.asarray(inputs["x"], np.float32)
    rope = np.asarray(inputs["rope_cache"], np.float32)
    wq = np.asarray(inputs["wq_w"], np.float32)
    bq = np.asarray(inputs["wq_b"], np.float32)
    wk = np.asarray(inputs["wk_w"], np.float32)
    bk = np.asarray(inputs["wk_b"], np.float32)
    wv = np.asarray(inputs["wv_w"], np.float32)
    bv = np.asarray(inputs["wv_b"], np.float32)
    wo = np.asarray(inputs["wo_w"], np.float32)
    sinks = np.asarray(inputs["sinks"], np.float32)

    xT = np.ascontiguousarray(x[0].T).astype(BF)            # [DIM, S]
    cosT = rope[:, :HD].T                                   # [64, S]
    sinT = rope[:, HD:].T
    cos2 = np.ascontiguousarray(np.concatenate([cosT, cosT], 0)).astype(BF)
    # sin_rot indexed by SOURCE partition: source rows hd in [0,32) land at
    # out rows hd+32 with +sin[hd+32]; source rows hd in [32,64) land at
    # out rows hd-32 with -sin[hd-32]. Duplicated for both heads per tile.
    sr = np.concatenate([sinT[32:64], -sinT[0:32]], 0)      # [64, S]
    sin_rot2 = np.ascontiguousarray(np.concatenate([sr, sr], 0)).astype(BF)
    tri = np.triu(np.ones((128, 128), BF))                  # mask[p, j] = j >= p
    ident = np.eye(HD, dtype=np.float32)
    ones_col = np.ones((128, 64), BF)
    ones_f = np.ones((128, 64), np.float32)

    in_maps = []
    for c in range(NCORES):
        qs = slice(c * QH * HD, (c + 1) * QH * HD)          # 256 q rows
        ks = slice(c * HD, (c + 1) * HD)                    # 64 kv rows
        # wproj columns: [q 256 | k 64 | v 64] = 384
        wproj = np.concatenate([wq[qs].T, wk[ks].T, wv[ks].T], axis=1)
        bcol = np.zeros((128, 3), np.float32)
        bcol[:, 0] = bq[qs][0:128]
        bcol[:, 1] = bq[qs][128:256]
        bcol[0:64, 2] = bk[ks]
        bcol[64:128, 2] = bv[ks]
        woT = np.ascontiguousarray(wo[:, qs].T).astype(BF)  # [256, DIM]
        esc = np.zeros((128, 1), np.float32)
        for h in range(QH):
            esc[32 * h, 0] = np.exp(sinks[c * QH + h])
        in_maps.append(
            {
                "xT": xT,
                "wproj": np.ascontiguousarray(wproj).astype(BF),
                "bproj": bcol,
                "cos2": cos2,
                "sinr2": sin_rot2,
                "woT": woT,
                "esinkc": esc,
                "tri": tri,
                "identf": ident,
                "onesb": ones_col,
                "onesf": ones_f,
            }
        )
    return in_maps


def build_nc(split_waits=True):
    nc = bass.Bass("TRN2", target_bir_lowering=False, debug=False, num_devices=NCORES)
    xT = nc.dram_tensor("xT", [DIM, S], BF16, kind="ExternalInput").ap()
    wproj = nc.dram_tensor("wproj", [DIM, 384], BF16, kind="ExternalInput").ap()
    bproj = nc.dram_tensor("bproj", [128, 3], F32, kind="ExternalInput").ap()
    cos2 = nc.dram_tensor("cos2", [128, S], BF16, kind="ExternalInput").ap()
    sinr2 = nc.dram_tensor("sinr2", [128, S], BF16, kind="ExternalInput").ap()
    woT = nc.dram_tensor("woT", [2 * 128, DIM], BF16, kind="ExternalInput").ap()
    esinkc = nc.dram_tensor("esinkc", [128, 1], F32, kind="ExternalInput").ap()
    tri = nc.dram_tensor("tri", [128, 128], BF16, kind="ExternalInput").ap()
    identf = nc.dram_tensor("identf", [HD, HD], F32, kind="ExternalInput").ap()
    onesb = nc.dram_tensor("onesb", [128, 64], BF16, kind="ExternalInput").ap()
    onesf = nc.dram_tensor("onesf", [128, 64], F32, kind="ExternalInput").ap()
    out = nc.dram_tensor("out", [S, DIM], BF16, kind="ExternalOutput").ap()

    with tile.TileContext(nc) as tc:
        with (
            tc.tile_pool(name="persist", bufs=1) as P,
            tc.tile_pool(name="ps_s", bufs=2, space="PSUM") as PS_S,
            tc.tile_pool(name="ps_p", bufs=1, space="PSUM") as PS_P,
            tc.tile_pool(name="ps_o", bufs=2, space="PSUM") as PS_O,
            tc.tile_pool(name="ps_d", bufs=1, space="PSUM") as PS_D,
            tc.tile_pool(name="tmp", bufs=2) as TMP,
            tc.tile_pool(name="ptp", bufs=6) as PT,
            tc.tile_pool(name="evp", bufs=4) as EV,
            tc.tile_pool(name="rnp", bufs=2) as RN,
        ):
            # ---- persistent tiles ----
            esink_t = P.tile([128, 1], F32, tag="esink")
            tri_t = P.tile([128, 128], BF16, tag="tri")
            wo_t = [P.tile([128, DIM], BF16, name=f"wo{i}", tag=f"wo{i}") for i in range(2)]
            qp = [P.tile([128, S], BF16, name=f"qp{i}", tag=f"qp{i}") for i in range(2)]
            kT2 = P.tile([128, S], BF16, tag="kT2")
            vTf = P.tile([64, S], F32, tag="vTf")
            vext = P.tile([128, NST * 64], BF16, tag="vext")
            outstk = [P.tile([128, S], BF16, name=f"os{i}", tag=f"os{i}") for i in range(2)]
            id_t = P.tile([HD, HD], F32, tag="idp")
            onesb_t = P.tile([128, 64], BF16, tag="onesb_t")
            onesf_t = P.tile([128, 64], F32, tag="onesf_t")
            bcol_t = P.tile([128, 3], F32, tag="bcol")
            cos_t = P.tile([128, S], BF16, tag="cos")
            sinr_t = P.tile([128, S], BF16, tag="sinr")
            scr = P.tile([1, 16], F32, tag="scr")
            x_t, w_t = [], []
            for dc in range(NDC):
                wt = P.tile([128, 384], BF16, name=f"w{dc}", tag=f"w{dc}")
                nc.gpsimd.dma_start(wt[:], wproj[dc * 128 : (dc + 1) * 128, :])
                w_t.append(wt)
            big = [P.tile([128, S], BF16, name=f"x{dc}", tag=f"x{dc}") for dc in range(NDC)]
            x_t = big
            _xq = [nc.sync, nc.scalar, nc.gpsimd]
            for dc in range(NDC):
                _xq[dc % 3].dma_start(x_t[dc][:], xT[dc * 128 : (dc + 1) * 128, :])
            nc.gpsimd.dma_start(bcol_t[:], bproj[:])
            nc.gpsimd.dma_start(cos_t[:], cos2[:])
            nc.gpsimd.dma_start(sinr_t[:], sinr2[:])
            nc.gpsimd.dma_start(id_t[:], identf[:])
            nc.gpsimd.dma_start(onesb_t[:], onesb[:])
            nc.gpsimd.dma_start(onesf_t[:], onesf[:])
            nc.gpsimd.dma_start(esink_t[:], esinkc[:])
            nc.gpsimd.dma_start(tri_t[:], tri[:])
            for i in range(2):
                nc.gpsimd.dma_start(wo_t[i][:], woT[i * 128 : (i + 1) * 128, :])
            # pull the ACT Exp/Ln table load off the critical path
            nc.scalar.activation(scr[0:1, 0:3], bcol_t[0:1, 0:3], AF.Exp)
            nc.scalar.activation(scr[0:1, 0:3], scr[0:1, 0:3], AF.Ln)
            nc.scalar.activation(scr[0:1, 0:3], scr[0:1, 0:3], AF.Identity)

            # ---------------- helpers (emit instructions) ----------------

            def rope_q(i, pp, ss):
                """Evict q tile i from psum pp (one fast read), then RoPE in
                bf16 on SBUF (2x DVE mode); frees the proj psum bank early."""
                u = TMP.tile([128, SBLK], BF16, name="u", tag="u")
                nc.scalar.activation(u[:], pp, AF.Identity, bias=bcol_t[:, i : i + 1])
                t1 = TMP.tile([128, SBLK], BF16, name="t1", tag="t1")
                nc.vector.tensor_tensor(t1[:], u[:], cos_t[:, ss], op=OP.mult)
                t2 = TMP.tile([128, SBLK], BF16, name="t2", tag="t2")
                for g in range(4):
                    d0 = 32 * g
                    s0 = d0 + 32 if g % 2 == 0 else d0 - 32
                    nc.vector.tensor_tensor(
                        t2[d0 : d0 + 32, :], u[s0 : s0 + 32, :],
                        sinr_t[s0 : s0 + 32, ss], op=OP.mult,
                    )
                nc.vector.tensor_tensor(qp[i][:, ss], t1[:], t2[:], op=OP.add)

            def rope_kv(pp, ss):
                uk = TMP.tile([64, SBLK], BF16, name="uk", tag="uk")
                nc.scalar.activation(uk[:], pp[0:64, :], AF.Identity, bias=bcol_t[0:64, 2:3])
                # v rows with bias, fp32 (transposed later on PE)
                nc.scalar.activation(vTf[:, ss], pp[64:128, :], AF.Identity, bias=bcol_t[64:128, 2:3])
                tk1 = TMP.tile([64, SBLK], BF16, name="tk1", tag="tk1")
                nc.vector.tensor_tensor(tk1[:], uk[:], cos_t[0:64, ss], op=OP.mult)
                tk2 = TMP.tile([64, SBLK], BF16, name="tk2", tag="tk2")
                nc.vector.tensor_tensor(
                    tk2[0:32, :], uk[32:64, :], sinr_t[32:64, ss], op=OP.mult
                )
                nc.vector.tensor_tensor(
                    tk2[32:64, :], uk[0:32, :], sinr_t[0:32, ss], op=OP.mult
                )
                nc.vector.tensor_tensor(kT2[0:64, ss], tk1[:], tk2[:], op=OP.add)
                nc.vector.tensor_copy(kT2[64:128, ss], kT2[0:64, ss])

            def transp_round(r):
                """Transpose this round's 4 v tiles into vext via the S ring."""
                tp = PS_S.tile([128, 1024], F32, name="tp", tag="s")
                for j in range(4):
                    t = 4 * r + j
                    nc.tensor.transpose(
                        tp[:, j * 64 : (j + 1) * 64],
                        vTf[:, t * 128 : (t + 1) * 128],
                        id_t[:],
                    )
                    nc.vector.tensor_copy(
                        vext[:, t * 64 : (t + 1) * 64], tp[:, j * 64 : (j + 1) * 64]
                    )

            pso_cur = [None]
            den_cur = [None]
            ptt_store = {}

            def s_iter(b, t):
                """Scores + exp for sk-tile t of block b (both head pairs)."""
                off = 128 * (t - 4 * b) if t >= 4 * b else 0
                n0 = b * SBLK + off
                ptts = []
                for pi in range(2):
                    sbt = PS_S.tile([128, 1024], F32, name="sbt", tag="s")
                    # lane0 at [off:512], lane1 packed at [512:1024-off] so the
                    # exp input region is contiguous (no stale-data gap)
                    for lane, (c0, c1) in enumerate([(off, 512), (512, 1024 - off)]):
                        nc.tensor.matmul(
                            sbt[:, c0:c1],
                            kT2[64 * lane : 64 * lane + 64, t * 128 : (t + 1) * 128],
                            qp[pi][64 * lane : 64 * lane + 64, n0 : (b + 1) * SBLK],
                            start=True, stop=True,
                            tile_position=(64 * lane, 0),
                        )
                    ptt = PT.tile([128, 1024], BF16, name="ptt", tag="pt")
                    nc.scalar.activation(
                        ptt[:, off : 1024 - off], sbt[:, off : 1024 - off],
                        AF.Exp, scale=SCALE,
                    )
                    if t >= 4 * b:
                        for c in (off, 512):
                            nc.vector.tensor_tensor(
                                ptt[:, c : c + 128], ptt[:, c : c + 128],
                                tri_t[:], op=OP.mult,
                            )
                    ptts.append(ptt)
                ptt_store[(b, t)] = ptts

            def pv_iter(b, t):
                """PV accumulation + denominators for sk-tile t of block b."""
                off = 128 * (t - 4 * b) if t >= 4 * b else 0
                ptts = ptt_store.pop((b, t))
                if t == 0:
                    pso_cur[0] = [
                        PS_O.tile([128, SBLK], F32, name=f"pso{pi}", tag="o")
                        for pi in range(2)
                    ]
                    den_cur[0] = PS_D.tile([128, SBLK], F32, name="den", tag="d")
                pso, den = pso_cur[0], den_cur[0]
                vx = vext[:, t * 64 : (t + 1) * 64]
                first = t == 0
                last = t == 4 * b + 3
                for pi in range(2):
                    nc.tensor.matmul(
                        pso[pi][0:64, off:SBLK],
                        vx, ptts[pi][:, off:512],
                        start=first, stop=last, tile_position=(0, 0),
                    )
                    # skip_group_check: the sim's zero-region bookkeeping
                    # mis-translates partition-offset APs (aliases partitions
                    # 8..71); flags here mirror the checked even-half matmul
                    nc.tensor.matmul(
                        pso[pi][64:128, off:SBLK],
                        vx, ptts[pi][:, 512 : 1024 - off],
                        start=first, stop=last, tile_position=(0, 64),
                        skip_group_check=True,
                    )
                for h in range(QH):
                    pi, odd = h // 2, h % 2
                    # M=32 (same cycles as M=1) so the whole den bank is
                    # initialized for the full-tile renorm read later
                    nc.tensor.matmul(
                        den[32 * h : 32 * h + 32, off:SBLK],
                        onesb_t[:, 0:32],
                        ptts[pi][:, 512 * odd + off * (1 - odd) : 512 + 512 * odd - off * odd],
                        start=first, stop=last, tile_position=(0, 32 * h),
                        skip_group_check=(h > 0),
                    )

            def renorm_tail(b):
                """r = sumexp + e^sink; outstk = pso / r (broadcast via K=1 mm)."""
                bs = slice(b * SBLK, (b + 1) * SBLK)
                pso, den = pso_cur[0], den_cur[0]
                rowb = RN.tile([128, SBLK], F32, name="rowb", tag="rowb")
                nc.vector.tensor_scalar_add(rowb[:], den[:], esink_t[:])
                lnr = RN.tile([128, SBLK], F32, name="lnr", tag="lnr")
                nc.scalar.activation(lnr[:], rowb[:], AF.Ln)
                rinv = RN.tile([128, SBLK], F32, name="rinv", tag="rinv")
                nc.scalar.activation(rinv[:], lnr[:], AF.Exp, scale=-1.0)
                rbt = PS_S.tile([128, 1024], F32, name="rbt", tag="s")
                for h in range(QH):
                    pi, odd = h // 2, h % 2
                    nc.tensor.matmul(
                        rbt[64 * odd : 64 * odd + 64, 512 * pi : 512 * pi + 512],
                        onesf_t[32 * h : 32 * h + 1, :], rinv[32 * h : 32 * h + 1, :],
                        start=True, stop=True, tile_position=(32 * h, 64 * odd),
                    )
                rbs = RN.tile([128, 1024], F32, name="rbs", tag="rbs")
                nc.vector.tensor_copy(rbs[:, 0:512], rbt[:, 0:512])
                nc.scalar.copy(rbs[:, 512:1024], rbt[:, 512:1024])
                for pi in range(2):
                    nc.vector.tensor_tensor(
                        outstk[pi][:, bs], pso[pi][:],
                        rbs[:, 512 * pi : 512 * pi + 512], op=OP.mult,
                    )

            def outproj_tile(st, dpair, split_cast=False):
                """Project sq tile st for output column pair dpair (2x512)."""
                psf = PS_S.tile([128, 1024], F32, name="psf", tag="s")
                for half in range(2):
                    db = 2 * dpair + half
                    ds = slice(db * SBLK, (db + 1) * SBLK)
                    nc.tensor.matmul(
                        psf[:, 512 * half : 512 * half + 512],
                        outstk[0][:, st * 128 : (st + 1) * 128], wo_t[0][:, ds],
                        start=True, stop=False,
                    )
                    nc.tensor.matmul(
                        psf[:, 512 * half : 512 * half + 512],
                        outstk[1][:, st * 128 : (st + 1) * 128], wo_t[1][:, ds],
                        start=False, stop=True,
                    )
                    ot = EV.tile([128, SBLK], BF16, name="ot", tag="ev")
                    if split_cast and half == 1:
                        nc.scalar.copy(ot[:], psf[:, 512 * half : 512 * half + 512])
                    else:
                        nc.vector.tensor_copy(ot[:], psf[:, 512 * half : 512 * half + 512])
                    _oq = nc.sync if half == 0 else nc.gpsimd
                    _oq.dma_start(out[st * 128 : (st + 1) * 128, ds], ot[:])

            def proj_group(g, ss, pp):
                c0 = 128 * g if g < 2 else 256
                c1 = c0 + 128
                for dc in range(NDC):
                    nc.tensor.matmul(
                        pp, w_t[dc][:, c0:c1], x_t[dc][:, ss],
                        start=(dc == 0), stop=(dc == NDC - 1),
                    )

            # ---------------- round 0: projection of block 0 ----------------
            ss0 = slice(0, SBLK)
            s_q = PS_S.tile([128, 1024], F32, name="s_q", tag="s")
            s_k = PS_S.tile([128, 1024], F32, name="s_k", tag="s")
            for dc in range(NDC):
                nc.tensor.matmul(
                    s_q[:, 0:512], w_t[dc][:, 0:128], x_t[dc][:, ss0],
                    start=(dc == 0), stop=(dc == NDC - 1),
                )
                nc.tensor.matmul(
                    s_q[:, 512:1024], w_t[dc][:, 128:256], x_t[dc][:, ss0],
                    start=(dc == 0), stop=(dc == NDC - 1),
                )
                nc.tensor.matmul(
                    s_k[:, 0:512], w_t[dc][:, 256:384], x_t[dc][:, ss0],
                    start=(dc == 0), stop=(dc == NDC - 1),
                )
            rope_q(0, s_q[:, 0:512], ss0)
            rope_q(1, s_q[:, 512:1024], ss0)
            rope_kv(s_k[:, 0:512], ss0)
            transp_round(0)
            s_iter(0, 0)
            s_iter(0, 1)

            # ------- rounds 1..3: proj r + attn r-1, score-lead pipeline -----
            for r in range(1, NSB):
                b = r - 1
                nt = 4 * b + 4
                ssr = slice(r * SBLK, (r + 1) * SBLK)

                fillers = []

                def mk_proj(g, ssr=ssr):
                    def f():
                        pp = PS_P.tile([128, SBLK], F32, name="pp", tag="p")
                        proj_group(g, ssr, pp[:])
                        if g < 2:
                            rope_q(g, pp[:], ssr)
                        else:
                            rope_kv(pp[:], ssr)
                    return f

                for g in range(3):
                    fillers.append(mk_proj(g))
                if b >= 1:
                    for st in range(4 * (b - 1), 4 * (b - 1) + 4):
                        for dp in range(2):
                            fillers.append(
                                lambda st=st, dp=dp: outproj_tile(st, dp)
                            )
                fi = 0
                for i in range(nt):
                    if i + 2 < nt:
                        s_iter(b, i + 2)
                    pv_iter(b, i)
                    while fi < len(fillers) and (fi + 1) * nt <= (i + 1) * len(fillers):
                        fillers[fi]()
                        fi += 1
                while fi < len(fillers):
                    fillers[fi]()
                    fi += 1
                renorm_tail(b)
                transp_round(r)
                s_iter(r, 0)
                s_iter(r, 1)

            # ------- block 3 early iters, interleaved with outproj(2) -------
            ofill = [
                (lambda st=st, dp=dp: outproj_tile(st, dp))
                for st in range(8, 12) for dp in range(2)
            ]
            fi = 0
            for i in range(12):
                if i + 2 < 16:
                    s_iter(3, i + 2)
                pv_iter(3, i)
                while fi < len(ofill) and (fi + 1) * 12 <= (i + 1) * len(ofill):
                    ofill[fi]()
                    fi += 1
            while fi < len(ofill):
                ofill[fi]()
                fi += 1

            # ---------------- round 4: finish block 3 ----------------
            for i in range(12, 16):
                if i + 2 < 16:
                    s_iter(3, i + 2)
                pv_iter(3, i)
            renorm_tail(3)
            for st in range(12, 16):
                for dp in range(2):
                    outproj_tile(st, dp, split_cast=True)

    _fix_range_clears(nc)
    if split_waits:
        _split_excess_waits(nc)
    return nc


_nc_cache = [None]


def kernel(**inputs):
    in_maps = prep_inputs(inputs)
    if _nc_cache[0] is None:
        _nc_cache[0] = build_nc()
    nc = _nc_cache[0]
    res = run_bass_kernel_spmd(nc, in_maps, list(range(NCORES)))
    acc = res.results[0]["out"].astype(np.float32)
    for i in range(1, NCORES):
        acc = acc + res.results[i]["out"].astype(np.float32)
    acc = acc + np.asarray(inputs["wo_b"], np.float32).reshape(1, DIM)
    return acc.reshape(B, S, DIM)
